# revision 7
# baseline (speedup 1.0000x reference)
"""Causal single-head attention (B=4, T=4096, C=1024, H=64) on 8 trn2 NeuronCores.

Sharding: core = (batch b = core//2, parity p = core%2). Each core owns the
interleaved context tiles {p, p+2, ...} of its batch (balanced under the causal
mask) and computes partial flash-attention (numerator + denominator) for ALL
queries of the batch; the host sums the two partials per batch and divides.

Device pipeline per core:
  load x_own [T/2, C] -> PE-transpose to x^T -> project [K^|Q^] col-packed
  (fp32r) -> pairwise AllGather of the k-projection (scores use S=k, G=q per
  the reference's K@Q^T convention) -> for each 512-query block: row-packed
  score matmuls -> exp on ACT (scale folded into the activation) -> causal
  masks (data-driven, multiplicative) -> PV matmuls accumulating
  [V|1]^T @ P^T in PSUM -> write O^T_aug [H+1, T] to DRAM.
"""

import sys

for _p in ("/root/.axon_site/_ro/trn_rl_repo", "/root/.axon_site/_ro/pypackages"):
    if _p not in sys.path:
        sys.path.append(_p)

import numpy as np

import concourse.bass as bass
import concourse.mybir as mybir
import concourse.tile as tile
from concourse import bacc
from concourse.bass_utils import run_bass_kernel_spmd
from concourse.masks import make_identity

B, T, C, H = 4, 4096, 1024, 64
N_CORES = 8
SCALE = C ** -0.5
F32 = mybir.dt.float32
F32R = mybir.dt.float32r
REPLICA_GROUPS = [[0, 1], [2, 3], [4, 5], [6, 7]]


def build_kernel(t_full=T):
    """Build the SPMD Bass/Tile program for sequence length t_full."""
    t_own = t_full // 2           # context rows owned by this core
    n_own = t_own // 128          # own 128-row s-tiles
    n_blk = t_full // 512         # 512-wide query blocks
    n_cchunk = C // 128           # contraction chunks of 128

    nc = bacc.Bacc("TRN2", target_bir_lowering=False, debug=False,
                   num_devices=N_CORES)

    x_d = nc.dram_tensor("x_own", [t_own, C], F32, kind="ExternalInput").ap()
    wk_d = nc.dram_tensor("wk", [C, H], F32, kind="ExternalInput").ap()
    wq_d = nc.dram_tensor("wq", [C, H], F32, kind="ExternalInput").ap()
    wv_d = nc.dram_tensor("wv", [C, H], F32, kind="ExternalInput").ap()
    bk_d = nc.dram_tensor("bk", [H], F32, kind="ExternalInput").ap()
    bq_d = nc.dram_tensor("bq", [H], F32, kind="ExternalInput").ap()
    bv_d = nc.dram_tensor("bv", [H], F32, kind="ExternalInput").ap()
    masks_d = nc.dram_tensor("masks", [2, 128, 512], F32,
                             kind="ExternalInput").ap()
    out_d = nc.dram_tensor("out_part", [H + 1, t_full], F32,
                           kind="ExternalOutput").ap()

    r = lambda ap: ap.bitcast(F32R)

    with tile.TileContext(nc) as tc:
        with (
            tc.tile_pool(name="persist", bufs=1) as pp,
            tc.tile_pool(name="dram", bufs=1, space="DRAM") as dp,
        ):
            # ---- persistent SBUF tensors ----
            xt = pp.tile([128, n_cchunk * t_own], F32R)      # x^T, chunk j at cols [t_own*j)
            kqT = pp.tile([128, t_own], F32R)                # rows 0:64 = S^T own, 64:128 = G^T own
            gt_lo = pp.tile([64, t_own], F32R)               # G^T copy at partitions 0:64
            stf = pp.tile([128, t_full], F32R)               # S^T full, duplicated row halves
            vT = pp.tile([64, t_own], F32)                  # V^T own
            v_sb = pp.tile([128, n_own * 65], F32R)          # V_aug tiles [128,65] per own s-tile
            wkq_sb = pp.tile([128, n_cchunk * 128], F32R)   # [wk|wq] fused stationary
            wv_sb = pp.tile([128, n_cchunk * H], F32R)
            bias_kq = pp.tile([128, 1], F32)
            bias_v = pp.tile([64, 1], F32)
            mask0 = pp.tile([128, 512], F32R)
            mask1 = pp.tile([128, 512], F32R)
            ident = pp.tile([128, 128], F32)

            make_identity(nc, ident[:, :])

            # weights: DRAM [C, H] -> SBUF; wk/wq fused side by side per chunk
            wkq_v = wkq_sb[:, :].rearrange("p (j s h) -> p j s h", s=2, h=H)
            nc.sync.dma_start(
                out=wkq_v[:, :, 0, :],
                in_=wk_d.rearrange("(j p) h -> p j h", p=128).bitcast(F32R))
            nc.sync.dma_start(
                out=wkq_v[:, :, 1, :],
                in_=wq_d.rearrange("(j p) h -> p j h", p=128).bitcast(F32R))
            nc.sync.dma_start(
                out=wv_sb[:, :].rearrange("p (j h) -> p j h", h=H),
                in_=wv_d.rearrange("(j p) h -> p j h", p=128).bitcast(F32R))
            nc.sync.dma_start(out=bias_kq[0:64, 0:1], in_=bk_d[:, None])
            nc.sync.dma_start(out=bias_kq[64:128, 0:1], in_=bq_d[:, None])
            nc.sync.dma_start(out=bias_v[:, 0:1], in_=bv_d[:, None])
            nc.sync.dma_start(out=mask0[:, :], in_=masks_d[0].bitcast(F32R))
            nc.sync.dma_start(out=mask1[:, :], in_=masks_d[1].bitcast(F32R))

            # ones column of V_aug (col 64 of each 65-wide slot); memset can't
            # emit f32r, so write in*0+1 via DVE which rounds on output
            nc.vector.tensor_scalar(
                v_sb[:, :].rearrange("p (i c) -> p i c", c=65)[:, :, 64],
                ident[:, 0:n_own], 0.0, 1.0,
                op0=mybir.AluOpType.mult, op1=mybir.AluOpType.add)

            # ---- phase A: load x_own and transpose into x^T ----
            with (
                tc.tile_pool(name="xnat", bufs=6) as xp,
                tc.tile_pool(name="psA", bufs=4, space="PSUM") as psa,
            ):
                n_grp = n_own // 4
                for g in range(n_grp):
                    nats = []
                    for u in range(4):
                        i = 4 * g + u
                        xn = xp.tile([128, C], F32, tag="xnat")
                        nc.sync.dma_start(out=xn[:, :],
                                          in_=x_d[128 * i:128 * (i + 1), :])
                        nats.append(xn)
                    for j in range(n_cchunk):
                        ps = psa.tile([128, 512], F32, tag="psA")
                        for u in range(4):
                            nc.tensor.transpose(
                                ps[:, 128 * u:128 * (u + 1)],
                                nats[u][:, 128 * j:128 * (j + 1)],
                                ident[:, :])
                        dst = xt[:, t_own * j + 512 * g: t_own * j + 512 * (g + 1)]
                        if g % 2 == 0:
                            nc.vector.tensor_copy(dst, ps[:, :])
                        else:
                            nc.scalar.copy(dst, ps[:, :])

            # ---- phase B: projections ----
            with tc.tile_pool(name="psB", bufs=2, space="PSUM") as psb:
                for tq in range(t_own // 512):
                    # fused [wk|wq] stationary: rows 0:64 = S^T, 64:128 = G^T
                    ps = psb.tile([128, 512], F32, tag="psKQ")
                    rhs = xt[:, :].rearrange("p (j t) -> p j t", t=t_own)[
                        :, :, 512 * tq:512 * (tq + 1)]
                    for j in range(n_cchunk):
                        nc.tensor.matmul(
                            ps[:, :], wkq_sb[:, 128 * j:128 * (j + 1)],
                            rhs[:, j], start=(j == 0), stop=(j == n_cchunk - 1))
                    nc.vector.tensor_scalar_add(
                        kqT[:, 512 * tq:512 * (tq + 1)], ps[:, :],
                        bias_kq[:, 0:1])

                for tq in range(t_own // 512):
                    ps = psb.tile([64, 512], F32, tag="psV")
                    rhs = xt[:, :].rearrange("p (j t) -> p j t", t=t_own)[
                        :, :, 512 * tq:512 * (tq + 1)]
                    for j in range(n_cchunk):
                        nc.tensor.matmul(
                            ps[:, :], wv_sb[:, H * j:H * (j + 1)],
                            rhs[:, j], start=(j == 0), stop=(j == n_cchunk - 1))
                    nc.vector.tensor_scalar_add(
                        vT[:, 512 * tq:512 * (tq + 1)], ps[:, :], bias_v[:, 0:1])

            # V^T -> V_aug natural tiles
            with tc.tile_pool(name="psVT", bufs=2, space="PSUM") as psvt:
                for i in range(n_own):
                    ps = psvt.tile([128, 64], F32, tag="psVT")
                    nc.tensor.transpose(
                        ps[:, :], vT[:, 128 * i:128 * (i + 1)],
                        ident[0:64, 0:64])
                    nc.vector.tensor_copy(v_sb[:, 65 * i:65 * i + 64], ps[:, :])

            # ---- phase C: pairwise AllGather of S^T (k-projection) ----
            cc_in = dp.tile([64, t_own], F32)
            cc_out = dp.tile([128, t_own], F32)
            nc.gpsimd.dma_start(cc_in[:, :], kqT[0:64, :].bitcast(F32))
            nc.gpsimd.collective_compute(
                "AllGather", mybir.AluOpType.bypass,
                replica_groups=REPLICA_GROUPS,
                ins=[cc_in[:, :].opt()],
                outs=[cc_out[:, :].opt()],
            )
            # interleave the two halves back to absolute tile order,
            # duplicated into both partition halves (for row-packed matmuls)
            for q in (0, 1):
                src = cc_out[64 * q:64 * (q + 1), :].rearrange(
                    "p (a c) -> p a c", c=128)
                for dh in (0, 1):
                    dst = stf[64 * dh:64 * (dh + 1), :].rearrange(
                        "p (a b c) -> p a b c", b=2, c=128)[:, :, q, :]
                    nc.sync.dma_start(out=dst, in_=src.bitcast(F32R))
            # duplicate G^T into partitions 0:64
            nc.sync.dma_start(gt_lo[:, :], kqT[64:128, :])

            # ---- phase D: flash attention main loop ----
            with (
                tc.tile_pool(name="psS", bufs=2, space="PSUM") as pss,
                tc.tile_pool(name="psO", bufs=2, space="PSUM") as pso,
                tc.tile_pool(name="ptp", bufs=3) as ptp,
                tc.tile_pool(name="outp", bufs=2) as outp,
            ):
                for tb in range(n_blk):
                    po = pso.tile([65, 512], F32, tag="psO")
                    for ip in range(tb + 1):
                        i0, i1 = 2 * ip, 2 * ip + 1
                        ps = pss.tile([128, 1024], F32, tag="psS")
                        pt = ptp.tile([128, 1024], F32R, tag="pt")
                        nc.tensor.matmul(
                            ps[:, 0:512],
                            gt_lo[:, 128 * i0:128 * (i0 + 1)],
                            stf[0:64, 512 * tb:512 * (tb + 1)],
                            start=True, stop=True, tile_position=(0, 0))
                        nc.tensor.matmul(
                            ps[:, 512:1024],
                            kqT[64:128, 128 * i1:128 * (i1 + 1)],
                            stf[64:128, 512 * tb:512 * (tb + 1)],
                            start=True, stop=True, tile_position=(64, 0))
                        nc.scalar.activation(
                            pt[:, :], ps[:, :],
                            mybir.ActivationFunctionType.Exp, scale=SCALE)
                        if ip == tb:
                            nc.gpsimd.tensor_mul(
                                pt[:, 0:512], pt[:, 0:512], mask0[:, :])
                            nc.gpsimd.tensor_mul(
                                pt[:, 512:1024], pt[:, 512:1024], mask1[:, :])
                        nc.tensor.matmul(
                            po[:, :], v_sb[:, 65 * i0:65 * (i0 + 1)],
                            pt[:, 0:512],
                            start=(ip == 0), stop=False)
                        nc.tensor.matmul(
                            po[:, :], v_sb[:, 65 * i1:65 * (i1 + 1)],
                            pt[:, 512:1024],
                            start=False, stop=(ip == tb))
                    ob = outp.tile([65, 512], F32, tag="ob")
                    nc.vector.tensor_copy(ob[:, :], po[:, :])
                    nc.sync.dma_start(
                        out=out_d[:, 512 * tb:512 * (tb + 1)], in_=ob[:, :])

    nc.compile()
    return nc


def make_core_inputs(x, Wk, bk, Wq, bq, Wv, bv, t_full=T):
    """Shard FULL inputs into the 8 per-core input dicts."""
    n_tiles = t_full // 128
    ins = []
    for core in range(N_CORES):
        b, p = core // 2, core % 2
        own = np.concatenate(
            [x[b, 128 * j:128 * (j + 1), :] for j in range(p, n_tiles, 2)],
            axis=0)
        masks = np.zeros((2, 128, 512), np.float32)
        for m in (0, 1):
            d = p + 2 * m
            rr = np.arange(128)[:, None]
            cc = np.arange(512)[None, :]
            masks[m] = (cc >= 128 * d + rr).astype(np.float32)
        ins.append({
            "x_own": np.ascontiguousarray(own, np.float32),
            "wk": np.asarray(Wk, np.float32), "wq": np.asarray(Wq, np.float32),
            "wv": np.asarray(Wv, np.float32),
            "bk": np.asarray(bk, np.float32), "bq": np.asarray(bq, np.float32),
            "bv": np.asarray(bv, np.float32),
            "masks": masks,
        })
    return ins


def combine_outputs(parts, t_full=T):
    """parts: list of 8 arrays [H+1, t_full] -> full output [B, t_full, H]."""
    out = np.empty((B, t_full, H), np.float32)
    for b in range(B):
        acc = parts[2 * b] + parts[2 * b + 1]
        out[b] = (acc[:H, :] / acc[H:H + 1, :]).T
    return out


_NC_CACHE = {}


def kernel(x, Wk, bk, Wq, bq, Wv, bv):
    x = np.asarray(x, np.float32)
    t_full = x.shape[1]
    if t_full not in _NC_CACHE:
        _NC_CACHE[t_full] = build_kernel(t_full)
    nc = _NC_CACHE[t_full]
    ins = make_core_inputs(x, Wk, bk, Wq, bq, Wv, bv, t_full)
    res = run_bass_kernel_spmd(nc, ins, list(range(N_CORES)))
    parts = [res.results[i]["out_part"] for i in range(N_CORES)]
    return combine_outputs(parts, t_full)


if __name__ == "__main__":
    rng = np.random.default_rng(0)
    x = rng.standard_normal((B, T, C), dtype=np.float32)
    Wk = rng.standard_normal((C, H), dtype=np.float32) * SCALE
    Wq = rng.standard_normal((C, H), dtype=np.float32) * SCALE
    Wv = rng.standard_normal((C, H), dtype=np.float32) * SCALE
    bk = rng.standard_normal(H).astype(np.float32) * 0.02
    bq = rng.standard_normal(H).astype(np.float32) * 0.02
    bv = rng.standard_normal(H).astype(np.float32) * 0.02
    out = kernel(x=x, Wk=Wk, bk=bk, Wq=Wq, bq=bq, Wv=Wv, bv=bv)
    print(out.shape, out.dtype)


# revision 11
# speedup vs baseline: 1.2452x; 1.2452x over previous
"""Causal single-head attention (B=4, T=4096, C=1024, H=64) on 8 trn2 NeuronCores.

Sharding: core = (batch b = core//2, parity p = core%2). Each core owns the
interleaved context tiles {p, p+2, ...} of its batch (balanced under the causal
mask) and computes partial flash-attention (numerator + denominator) for ALL
queries of the batch; the host sums the two partials per batch and divides.

Device pipeline per core:
  load x_own [T/2, C] -> PE-transpose to x^T (fp32r) -> project [K^|Q^] with a
  fused [wk|wq] stationary (fp32r) -> pairwise chunked AllGather of the bf16
  k-projection (scores use S=k, G=q per the reference's K@Q^T convention) ->
  for each 512-query block: row-packed bf16 score matmuls -> exp on ACT (scale
  folded into the activation) -> data-driven causal masks on DVE -> bf16 PV
  matmuls accumulating [V|1]^T @ P^T in PSUM -> write O^T_aug [H+1, T].

Query columns are processed in by-rank order (rank0 tiles | rank1 tiles per
512-block); the host maps them back to absolute order in combine_outputs.
"""

import sys

for _p in ("/root/.axon_site/_ro/trn_rl_repo", "/root/.axon_site/_ro/pypackages"):
    if _p not in sys.path:
        sys.path.append(_p)

import ml_dtypes
import numpy as np

import concourse.bass as bass
import concourse.mybir as mybir
import concourse.tile as tile
from concourse import bacc
from concourse.bass_utils import run_bass_kernel_spmd
from concourse.masks import make_identity

B, T, C, H = 4, 4096, 1024, 64
N_CORES = 8
SCALE = C ** -0.5
F32 = mybir.dt.float32
F32R = mybir.dt.float32r
BF16 = mybir.dt.bfloat16
REPLICA_GROUPS = [[0, 1], [2, 3], [4, 5], [6, 7]]
SUB2ABS = [0, 2, 1, 3]      # 128-col sub-tile -> abs tile offset within 512-blk


def build_kernel(t_full=T):
    """Build the SPMD Bass/Tile program for sequence length t_full."""
    t_own = t_full // 2           # context rows owned by this core
    n_own = t_own // 128          # own 128-row s-tiles
    n_blk = t_full // 512         # 512-wide query blocks
    n_cchunk = C // 128           # contraction chunks of 128
    n_grp = n_own // 4            # x load groups of 4 tiles
    n_tq = t_own // 512           # projection column blocks

    nc = bacc.Bacc("TRN2", target_bir_lowering=False, debug=False,
                   num_devices=N_CORES)

    x_d = nc.dram_tensor("x_own", [t_own, C], F32, kind="ExternalInput").ap()
    wk_d = nc.dram_tensor("wk", [C, H], F32, kind="ExternalInput").ap()
    wq_d = nc.dram_tensor("wq", [C, H], F32, kind="ExternalInput").ap()
    wv_d = nc.dram_tensor("wv", [C, H], F32, kind="ExternalInput").ap()
    bk_d = nc.dram_tensor("bk", [H], F32, kind="ExternalInput").ap()
    bq_d = nc.dram_tensor("bq", [H], F32, kind="ExternalInput").ap()
    bv_d = nc.dram_tensor("bv", [H], F32, kind="ExternalInput").ap()
    masks_d = nc.dram_tensor("masks", [2, 128, 512], BF16,
                             kind="ExternalInput").ap()
    out_d = nc.dram_tensor("out_part", [H + 1, t_full], F32,
                           kind="ExternalOutput").ap()

    with tile.TileContext(nc) as tc:
        with (
            tc.tile_pool(name="persist", bufs=1) as pp,
            tc.tile_pool(name="dram", bufs=1, space="DRAM") as dp,
            tc.tile_pool(name="xg", bufs=2) as xp,
            tc.tile_pool(name="psA", bufs=2, space="PSUM") as psa,
            tc.tile_pool(name="psB", bufs=1, space="PSUM") as psb,
            tc.tile_pool(name="psS", bufs=2, space="PSUM") as pss,
            tc.tile_pool(name="psO", bufs=1, space="PSUM") as pso,
            tc.tile_pool(name="ptp", bufs=3) as ptp,
            tc.tile_pool(name="outp", bufs=2) as outp,
        ):
            # ---- persistent SBUF tensors ----
            xt = pp.tile([128, n_cchunk * t_own], F32R)   # x^T, chunk j at cols t_own*j
            kqT = pp.tile([128, t_own], BF16)             # 0:64 = S^T own, 64:128 = G^T own
            gt_lo = pp.tile([64, t_own], BF16)            # G^T copy at partitions 0:64
            stf = pp.tile([128, t_full], BF16)            # S^T by rank, dup row halves
            vT = pp.tile([64, t_own], F32)                # V^T own
            v_sb = pp.tile([128, n_own * 65], BF16)       # V_aug tiles [128,65]
            wkq_sb = pp.tile([128, n_cchunk * 128], F32R)  # [wk|wq] fused stationary
            wv_sb = pp.tile([128, n_cchunk * H], F32R)
            bias_kq = pp.tile([128, 1], F32)
            bias_v = pp.tile([64, 1], F32)
            mask0 = pp.tile([128, 512], BF16)
            mask1 = pp.tile([128, 512], BF16)
            ident = pp.tile([128, 128], F32)

            make_identity(nc, ident[:, :])

            # weights: DRAM [C, H] -> SBUF; wk/wq fused side by side per chunk
            wkq_v = wkq_sb[:, :].rearrange("p (j s h) -> p j s h", s=2, h=H)
            nc.sync.dma_start(
                out=wkq_v[:, :, 0, :],
                in_=wk_d.rearrange("(j p) h -> p j h", p=128).bitcast(F32R))
            nc.sync.dma_start(
                out=wkq_v[:, :, 1, :],
                in_=wq_d.rearrange("(j p) h -> p j h", p=128).bitcast(F32R))
            nc.sync.dma_start(
                out=wv_sb[:, :].rearrange("p (j h) -> p j h", h=H),
                in_=wv_d.rearrange("(j p) h -> p j h", p=128).bitcast(F32R))
            nc.sync.dma_start(out=bias_kq[0:64, 0:1], in_=bk_d[:, None])
            nc.sync.dma_start(out=bias_kq[64:128, 0:1], in_=bq_d[:, None])
            nc.sync.dma_start(out=bias_v[:, 0:1], in_=bv_d[:, None])
            nc.sync.dma_start(out=mask0[:, :], in_=masks_d[0])
            nc.sync.dma_start(out=mask1[:, :], in_=masks_d[1])

            # ones column of V_aug (col 64 of each 65-wide slot)
            nc.vector.tensor_scalar(
                v_sb[:, :].rearrange("p (i c) -> p i c", c=65)[:, :, 64],
                ident[:, 0:n_own], 0.0, 1.0,
                op0=mybir.AluOpType.mult, op1=mybir.AluOpType.add)

            cc_in = [dp.tile([64, t_own // 2], BF16, name=f"cci{c}",
                             tag=f"cci{c}") for c in range(2)]
            cc_out = [dp.tile([128, t_own // 2], BF16, name=f"cco{c}",
                              tag=f"cco{c}") for c in range(2)]

            def load_transpose_group(g):
                xg = xp.tile([128, 4 * C], F32, tag="xg")
                nc.sync.dma_start(
                    out=xg[:, :].rearrange("p (u c) -> p u c", c=C),
                    in_=x_d[512 * g:512 * (g + 1), :].rearrange(
                        "(u p) c -> p u c", p=128))
                for j in range(n_cchunk):
                    ps = psa.tile([128, 512], F32, tag="psA")
                    for u in range(4):
                        nc.tensor.transpose(
                            ps[:, 128 * u:128 * (u + 1)],
                            xg[:, 1024 * u + 128 * j:1024 * u + 128 * (j + 1)],
                            ident[:, :])
                    dst = xt[:, t_own * j + 512 * g: t_own * j + 512 * (g + 1)]
                    if g % 2 == 0:
                        nc.vector.tensor_copy(dst, ps[:, :])
                    else:
                        nc.scalar.copy(dst, ps[:, :])

            def project_kq(tq):
                ps = psb.tile([128, 512], F32, tag="psB")
                rhs = xt[:, :].rearrange("p (j t) -> p j t", t=t_own)[
                    :, :, 512 * tq:512 * (tq + 1)]
                for j in range(n_cchunk):
                    nc.tensor.matmul(
                        ps[:, :], wkq_sb[:, 128 * j:128 * (j + 1)],
                        rhs[:, j], start=(j == 0), stop=(j == n_cchunk - 1))
                nc.vector.tensor_scalar_add(
                    kqT[:, 512 * tq:512 * (tq + 1)], ps[:, :], bias_kq[:, 0:1])

            def project_v(tq):
                ps = psb.tile([64, 512], F32, tag="psB")
                rhs = xt[:, :].rearrange("p (j t) -> p j t", t=t_own)[
                    :, :, 512 * tq:512 * (tq + 1)]
                for j in range(n_cchunk):
                    nc.tensor.matmul(
                        ps[:, :], wv_sb[:, H * j:H * (j + 1)],
                        rhs[:, j], start=(j == 0), stop=(j == n_cchunk - 1))
                nc.vector.tensor_scalar_add(
                    vT[:, 512 * tq:512 * (tq + 1)], ps[:, :], bias_v[:, 0:1])

            def v_transpose(i):
                ps = psb.tile([128, 64], F32, tag="psB")
                nc.tensor.transpose(
                    ps[:, :], vT[:, 128 * i:128 * (i + 1)], ident[0:64, 0:64])
                nc.vector.tensor_copy(v_sb[:, 65 * i:65 * i + 64], ps[:, :])

            def gather_chunk(ch):
                w = t_own // 2
                nc.gpsimd.dma_start(cc_in[ch][:, :],
                                    kqT[0:64, w * ch:w * (ch + 1)])
                nc.gpsimd.collective_compute(
                    "AllGather", mybir.AluOpType.bypass,
                    replica_groups=REPLICA_GROUPS,
                    ins=[cc_in[ch][:, :].opt()],
                    outs=[cc_out[ch][:, :].opt()],
                )
                # stf: [rank0 t_own | rank1 t_own], both partition halves
                for q in (0, 1):
                    for dh in (0, 1):
                        nc.sync.dma_start(
                            out=stf[64 * dh:64 * (dh + 1),
                                    t_own * q + w * ch:t_own * q + w * (ch + 1)],
                            in_=cc_out[ch][64 * q:64 * (q + 1), :])
                # G^T duplicate into partitions 0:64
                nc.sync.dma_start(gt_lo[:, w * ch:w * (ch + 1)],
                                  kqT[64:128, w * ch:w * (ch + 1)])

            # ---- phases: pipeline loads/projections with chunked gathers ----
            half_grp = max(n_grp // 2, 1)
            tq_split = max(n_tq // 2, 1)
            for g in range(half_grp):
                load_transpose_group(g)
            for tq in range(tq_split):
                project_kq(tq)
            gather_chunk(0)
            for tq in range(tq_split):
                project_v(tq)
            for i in range(n_own // 2):
                v_transpose(i)
            for g in range(half_grp, n_grp):
                load_transpose_group(g)
            for tq in range(tq_split, n_tq):
                project_kq(tq)
            gather_chunk(1)
            for tq in range(tq_split, n_tq):
                project_v(tq)
            for i in range(n_own // 2, n_own):
                v_transpose(i)

            # ---- flash attention main loop ----
            stf_lo = stf[0:64, :].rearrange("p (h t) -> p h t", h=2)
            stf_hi = stf[64:128, :].rearrange("p (h t) -> p h t", h=2)
            for tb in range(n_blk):
                po = pso.tile([65, 512], F32, tag="psO")
                for ip in range(tb + 1):
                    i0, i1 = 2 * ip, 2 * ip + 1
                    ps = pss.tile([128, 1024], F32, tag="psS")
                    pt = ptp.tile([128, 1024], BF16, tag="pt")
                    nc.tensor.matmul(
                        ps[:, 0:512],
                        gt_lo[:, 128 * i0:128 * (i0 + 1)],
                        stf_lo[:, :, 256 * tb:256 * (tb + 1)],
                        start=True, stop=True, tile_position=(0, 0))
                    nc.tensor.matmul(
                        ps[:, 512:1024],
                        kqT[64:128, 128 * i1:128 * (i1 + 1)],
                        stf_hi[:, :, 256 * tb:256 * (tb + 1)],
                        start=True, stop=True, tile_position=(64, 0))
                    nc.scalar.activation(
                        pt[:, :], ps[:, :],
                        mybir.ActivationFunctionType.Exp, scale=SCALE)
                    if ip == tb:
                        nc.vector.tensor_mul(
                            pt[:, 0:512], pt[:, 0:512], mask0[:, :])
                        nc.vector.tensor_mul(
                            pt[:, 512:1024], pt[:, 512:1024], mask1[:, :])
                    nc.tensor.matmul(
                        po[:, :], v_sb[:, 65 * i0:65 * (i0 + 1)],
                        pt[:, 0:512], start=(ip == 0), stop=False)
                    nc.tensor.matmul(
                        po[:, :], v_sb[:, 65 * i1:65 * (i1 + 1)],
                        pt[:, 512:1024], start=False, stop=(ip == tb))
                ob = outp.tile([65, 512], F32, tag="ob")
                nc.vector.tensor_copy(ob[:, :], po[:, :])
                nc.sync.dma_start(
                    out=out_d[:, 512 * tb:512 * (tb + 1)], in_=ob[:, :])

    nc.compile()
    return nc


def make_core_inputs(x, Wk, bk, Wq, bq, Wv, bv, t_full=T):
    """Shard FULL inputs into the 8 per-core input dicts."""
    n_tiles = t_full // 128
    ins = []
    for core in range(N_CORES):
        b, p = core // 2, core % 2
        own = np.concatenate(
            [x[b, 128 * j:128 * (j + 1), :] for j in range(p, n_tiles, 2)],
            axis=0)
        # mask[m][r, c]: s-tile (local parity m, abs tile 4tb+2m+p) vs query
        # sub-tile c//128 (abs tile 4tb + SUB2ABS[c//128]); valid iff s <= t
        masks = np.zeros((2, 128, 512), np.float32)
        rr = np.arange(128)[:, None]
        for m in (0, 1):
            for sub in range(4):
                cz = np.arange(128)[None, :]
                s_abs = 128 * (2 * m + p) + rr
                t_abs = 128 * SUB2ABS[sub] + cz
                masks[m, :, 128 * sub:128 * (sub + 1)] = (s_abs <= t_abs)
        ins.append({
            "x_own": np.ascontiguousarray(own, np.float32),
            "wk": np.asarray(Wk, np.float32), "wq": np.asarray(Wq, np.float32),
            "wv": np.asarray(Wv, np.float32),
            "bk": np.asarray(bk, np.float32), "bq": np.asarray(bq, np.float32),
            "bv": np.asarray(bv, np.float32),
            "masks": masks.astype(ml_dtypes.bfloat16),
        })
    return ins


def _col_perm(t_full):
    """stored column -> absolute t index (same for every core)."""
    perm = np.empty(t_full, np.int64)
    for tb in range(t_full // 512):
        for sub in range(4):
            a = 128 * (4 * tb + SUB2ABS[sub])
            s = 512 * tb + 128 * sub
            perm[s:s + 128] = np.arange(a, a + 128)
    return perm


def combine_outputs(parts, t_full=T):
    """parts: list of 8 arrays [H+1, t_full] -> full output [B, t_full, H]."""
    perm = _col_perm(t_full)
    out = np.empty((B, t_full, H), np.float32)
    for b in range(B):
        acc = parts[2 * b] + parts[2 * b + 1]
        res = acc[:H, :] / acc[H:H + 1, :]
        out[b][perm] = res.T
    return out


_NC_CACHE = {}


def kernel(x, Wk, bk, Wq, bq, Wv, bv):
    x = np.asarray(x, np.float32)
    t_full = x.shape[1]
    if t_full not in _NC_CACHE:
        _NC_CACHE[t_full] = build_kernel(t_full)
    nc = _NC_CACHE[t_full]
    ins = make_core_inputs(x, Wk, bk, Wq, bq, Wv, bv, t_full)
    res = run_bass_kernel_spmd(nc, ins, list(range(N_CORES)))
    parts = [res.results[i]["out_part"] for i in range(N_CORES)]
    return combine_outputs(parts, t_full)


if __name__ == "__main__":
    rng = np.random.default_rng(0)
    x = rng.standard_normal((B, T, C), dtype=np.float32)
    Wk = rng.standard_normal((C, H), dtype=np.float32) * SCALE
    Wq = rng.standard_normal((C, H), dtype=np.float32) * SCALE
    Wv = rng.standard_normal((C, H), dtype=np.float32) * SCALE
    bk = rng.standard_normal(H).astype(np.float32) * 0.02
    bq = rng.standard_normal(H).astype(np.float32) * 0.02
    bv = rng.standard_normal(H).astype(np.float32) * 0.02
    out = kernel(x=x, Wk=Wk, bk=bk, Wq=Wq, bq=bq, Wv=Wv, bv=bv)
    print(out.shape, out.dtype)


# revision 13
# speedup vs baseline: 1.2956x; 1.0405x over previous
"""Causal single-head attention (B=4, T=4096, C=1024, H=64) on 8 trn2 NeuronCores.

Sharding: core = (batch b = core//2, parity p = core%2). Each core owns the
interleaved context tiles {p, p+2, ...} of its batch (balanced under the causal
mask) and computes partial flash-attention (numerator + denominator) for ALL
queries of the batch; the host sums the two partials per batch and divides.

Device pipeline per core:
  load x_own [T/2, C] -> PE-transpose to x^T (fp32r) -> project [K^|Q^] with a
  fused [wk|wq] stationary (fp32r) -> pairwise chunked AllGather of the bf16
  k-projection (scores use S=k, G=q per the reference's K@Q^T convention) ->
  for each 512-query block: row-packed bf16 score matmuls -> exp on ACT (scale
  folded into the activation) -> data-driven causal masks on DVE -> bf16 PV
  matmuls accumulating [V|1]^T @ P^T in PSUM -> write O^T_aug [H+1, T].

Query columns are processed in by-rank order (rank0 tiles | rank1 tiles per
512-block); the host maps them back to absolute order in combine_outputs.
"""

import sys

for _p in ("/root/.axon_site/_ro/trn_rl_repo", "/root/.axon_site/_ro/pypackages"):
    if _p not in sys.path:
        sys.path.append(_p)

import ml_dtypes
import numpy as np

import concourse.bass as bass
import concourse.mybir as mybir
import concourse.tile as tile
from concourse import bacc
from concourse.bass_utils import run_bass_kernel_spmd
from concourse.masks import make_identity

B, T, C, H = 4, 4096, 1024, 64
N_CORES = 8
SCALE = C ** -0.5
F32 = mybir.dt.float32
F32R = mybir.dt.float32r
BF16 = mybir.dt.bfloat16
REPLICA_GROUPS = [[0, 1], [2, 3], [4, 5], [6, 7]]
SUB2ABS = [0, 2, 1, 3]      # 128-col sub-tile -> abs tile offset within 512-blk


def build_kernel(t_full=T):
    """Build the SPMD Bass/Tile program for sequence length t_full."""
    t_own = t_full // 2           # context rows owned by this core
    n_own = t_own // 128          # own 128-row s-tiles
    n_blk = t_full // 512         # 512-wide query blocks
    n_cchunk = C // 128           # contraction chunks of 128
    n_grp = n_own // 4            # x load groups of 4 tiles
    n_tq = t_own // 512           # projection column blocks

    nc = bacc.Bacc("TRN2", target_bir_lowering=False, debug=False,
                   num_devices=N_CORES)

    x_d = nc.dram_tensor("x_own", [t_own, C], F32, kind="ExternalInput").ap()
    wk_d = nc.dram_tensor("wk", [C, H], F32, kind="ExternalInput").ap()
    wq_d = nc.dram_tensor("wq", [C, H], F32, kind="ExternalInput").ap()
    wv_d = nc.dram_tensor("wv", [C, H], F32, kind="ExternalInput").ap()
    bk_d = nc.dram_tensor("bk", [H], F32, kind="ExternalInput").ap()
    bq_d = nc.dram_tensor("bq", [H], F32, kind="ExternalInput").ap()
    bv_d = nc.dram_tensor("bv", [H], F32, kind="ExternalInput").ap()
    masks_d = nc.dram_tensor("masks", [2, 128, 512], BF16,
                             kind="ExternalInput").ap()
    out_d = nc.dram_tensor("out_part", [H + 1, t_full], F32,
                           kind="ExternalOutput").ap()

    with tile.TileContext(nc) as tc:
        with (
            tc.tile_pool(name="persist", bufs=1) as pp,
            tc.tile_pool(name="dram", bufs=1, space="DRAM") as dp,
            tc.tile_pool(name="xg", bufs=4) as xp,
            tc.tile_pool(name="psA", bufs=1, space="PSUM") as psa,
            tc.tile_pool(name="psB", bufs=1, space="PSUM") as psb,
            tc.tile_pool(name="psS", bufs=2, space="PSUM") as pss,
            tc.tile_pool(name="psOA", bufs=1, space="PSUM") as psoa,
            tc.tile_pool(name="psOB", bufs=1, space="PSUM") as psob,
            tc.tile_pool(name="ptp", bufs=4) as ptp,
            tc.tile_pool(name="outp", bufs=2) as outp,
        ):
            # ---- persistent SBUF tensors ----
            xt = pp.tile([128, n_cchunk * t_own], F32R)   # x^T, chunk j at cols t_own*j
            kqT = pp.tile([128, t_own], BF16)             # 0:64 = S^T own, 64:128 = G^T own
            gt_lo = pp.tile([64, t_own], BF16)            # G^T copy at partitions 0:64
            stf = pp.tile([128, t_full], BF16)            # S^T by rank, dup row halves
            vT = pp.tile([64, t_own], F32)                # V^T own
            v_sb = pp.tile([128, n_own * 65], BF16)       # V_aug tiles [128,65]
            wkq_sb = pp.tile([128, n_cchunk * 128], F32R)  # [wk|wq] fused stationary
            wv_sb = pp.tile([128, n_cchunk * H], F32R)
            bias_kq = pp.tile([128, 1], F32)
            bias_v = pp.tile([64, 1], F32)
            mask0 = pp.tile([128, 512], BF16)
            mask1 = pp.tile([128, 512], BF16)
            ident = pp.tile([128, 128], F32)

            make_identity(nc, ident[:, :])

            # weights/masks/biases go on the scalar HWDGE queue so their tiny
            # descriptors don't delay the x stream on the sync queue
            wkq_v = wkq_sb[:, :].rearrange("p (j s h) -> p j s h", s=2, h=H)
            nc.scalar.dma_start(
                out=wkq_v[:, :, 0, :],
                in_=wk_d.rearrange("(j p) h -> p j h", p=128).bitcast(F32R))
            nc.scalar.dma_start(
                out=wkq_v[:, :, 1, :],
                in_=wq_d.rearrange("(j p) h -> p j h", p=128).bitcast(F32R))
            nc.scalar.dma_start(
                out=wv_sb[:, :].rearrange("p (j h) -> p j h", h=H),
                in_=wv_d.rearrange("(j p) h -> p j h", p=128).bitcast(F32R))
            nc.scalar.dma_start(out=bias_kq[0:64, 0:1], in_=bk_d[:, None])
            nc.scalar.dma_start(out=bias_kq[64:128, 0:1], in_=bq_d[:, None])
            nc.scalar.dma_start(out=bias_v[:, 0:1], in_=bv_d[:, None])
            nc.scalar.dma_start(out=mask0[:, :], in_=masks_d[0])
            nc.scalar.dma_start(out=mask1[:, :], in_=masks_d[1])

            # tiny warmup collective: absorbs TOPSP/mesh startup latency
            # while the x stream loads
            ccw_i = dp.tile([64, 16], BF16, name="ccwi", tag="ccwi")
            ccw_o = dp.tile([128, 16], BF16, name="ccwo", tag="ccwo")
            nc.gpsimd.dma_start(ccw_i[:, :], mask0[0:64, 0:16])
            nc.gpsimd.collective_compute(
                "AllGather", mybir.AluOpType.bypass,
                replica_groups=REPLICA_GROUPS,
                ins=[ccw_i[:, :].opt()],
                outs=[ccw_o[:, :].opt()],
            )

            # ones column of V_aug (col 64 of each 65-wide slot)
            nc.vector.tensor_scalar(
                v_sb[:, :].rearrange("p (i c) -> p i c", c=65)[:, :, 64],
                ident[:, 0:n_own], 0.0, 1.0,
                op0=mybir.AluOpType.mult, op1=mybir.AluOpType.add)

            cc_in = [dp.tile([64, t_own // 2], BF16, name=f"cci{c}",
                             tag=f"cci{c}") for c in range(2)]
            cc_out = [dp.tile([128, t_own // 2], BF16, name=f"cco{c}",
                              tag=f"cco{c}") for c in range(2)]

            xgs = {}

            def load_group(g):
                xg = xp.tile([128, 4 * C], F32, name=f"xg{g}", tag="xg")
                nc.sync.dma_start(
                    out=xg[:, :].rearrange("p (u c) -> p u c", c=C),
                    in_=x_d[512 * g:512 * (g + 1), :].rearrange(
                        "(u p) c -> p u c", p=128))
                xgs[g] = xg

            def transpose_group(g):
                xg = xgs[g]
                for j in range(n_cchunk):
                    ps = psa.tile([128, 512], F32, tag="psA")
                    for u in range(4):
                        nc.tensor.transpose(
                            ps[:, 128 * u:128 * (u + 1)],
                            xg[:, 1024 * u + 128 * j:1024 * u + 128 * (j + 1)],
                            ident[:, :])
                    dst = xt[:, t_own * j + 512 * g: t_own * j + 512 * (g + 1)]
                    if g % 2 == 0:
                        nc.vector.tensor_copy(dst, ps[:, :])
                    else:
                        nc.scalar.copy(dst, ps[:, :])

            def project_kq(tq):
                ps = psb.tile([128, 512], F32, tag="psB")
                rhs = xt[:, :].rearrange("p (j t) -> p j t", t=t_own)[
                    :, :, 512 * tq:512 * (tq + 1)]
                for j in range(n_cchunk):
                    nc.tensor.matmul(
                        ps[:, :], wkq_sb[:, 128 * j:128 * (j + 1)],
                        rhs[:, j], start=(j == 0), stop=(j == n_cchunk - 1))
                nc.vector.tensor_scalar_add(
                    kqT[:, 512 * tq:512 * (tq + 1)], ps[:, :], bias_kq[:, 0:1])

            def project_v(tq):
                ps = psb.tile([64, 512], F32, tag="psB")
                rhs = xt[:, :].rearrange("p (j t) -> p j t", t=t_own)[
                    :, :, 512 * tq:512 * (tq + 1)]
                for j in range(n_cchunk):
                    nc.tensor.matmul(
                        ps[:, :], wv_sb[:, H * j:H * (j + 1)],
                        rhs[:, j], start=(j == 0), stop=(j == n_cchunk - 1))
                nc.vector.tensor_scalar_add(
                    vT[:, 512 * tq:512 * (tq + 1)], ps[:, :], bias_v[:, 0:1])

            def v_transpose(i):
                ps = psb.tile([128, 64], F32, tag="psB")
                nc.tensor.transpose(
                    ps[:, :], vT[:, 128 * i:128 * (i + 1)], ident[0:64, 0:64])
                nc.vector.tensor_copy(v_sb[:, 65 * i:65 * i + 64], ps[:, :])

            def gather_chunk(ch):
                w = t_own // 2
                nc.gpsimd.dma_start(cc_in[ch][:, :],
                                    kqT[0:64, w * ch:w * (ch + 1)])
                nc.gpsimd.collective_compute(
                    "AllGather", mybir.AluOpType.bypass,
                    replica_groups=REPLICA_GROUPS,
                    ins=[cc_in[ch][:, :].opt()],
                    outs=[cc_out[ch][:, :].opt()],
                )
                # stf: [rank0 t_own | rank1 t_own], both partition halves
                for q in (0, 1):
                    for dh in (0, 1):
                        nc.sync.dma_start(
                            out=stf[64 * dh:64 * (dh + 1),
                                    t_own * q + w * ch:t_own * q + w * (ch + 1)],
                            in_=cc_out[ch][64 * q:64 * (q + 1), :])
                # G^T duplicate into partitions 0:64
                nc.sync.dma_start(gt_lo[:, w * ch:w * (ch + 1)],
                                  kqT[64:128, w * ch:w * (ch + 1)])

            # ---- phases: pipeline loads/projections with chunked gathers ----
            half_grp = max(n_grp // 2, 1)
            tq_split = max(n_tq // 2, 1)
            for g in range(n_grp):
                load_group(g)
            for g in range(half_grp):
                transpose_group(g)
            for tq in range(tq_split):
                project_kq(tq)
            gather_chunk(0)
            for tq in range(tq_split):
                project_v(tq)
            for i in range(n_own // 2):
                v_transpose(i)
            for g in range(half_grp, n_grp):
                transpose_group(g)
            for tq in range(tq_split, n_tq):
                project_kq(tq)
            gather_chunk(1)
            for tq in range(tq_split, n_tq):
                project_v(tq)
            for i in range(n_own // 2, n_own):
                v_transpose(i)

            # ---- flash attention main loop ----
            stf_lo = stf[0:64, :].rearrange("p (h t) -> p h t", h=2)
            stf_hi = stf[64:128, :].rearrange("p (h t) -> p h t", h=2)
            for tb in range(n_blk):
                poa = psoa.tile([65, 512], F32, tag="psOA")
                pob = psob.tile([65, 512], F32, tag="psOB")
                for ip in range(tb + 1):
                    i0, i1 = 2 * ip, 2 * ip + 1
                    ps = pss.tile([128, 1024], F32, tag="psS")
                    pt = ptp.tile([128, 1024], BF16, tag="pt")
                    nc.tensor.matmul(
                        ps[:, 0:512],
                        gt_lo[:, 128 * i0:128 * (i0 + 1)],
                        stf_lo[:, :, 256 * tb:256 * (tb + 1)],
                        start=True, stop=True, tile_position=(0, 0))
                    nc.tensor.matmul(
                        ps[:, 512:1024],
                        kqT[64:128, 128 * i1:128 * (i1 + 1)],
                        stf_hi[:, :, 256 * tb:256 * (tb + 1)],
                        start=True, stop=True, tile_position=(64, 0))
                    nc.scalar.activation(
                        pt[:, :], ps[:, :],
                        mybir.ActivationFunctionType.Exp, scale=SCALE)
                    if ip == tb:
                        nc.vector.tensor_mul(
                            pt[:, 0:512], pt[:, 0:512], mask0[:, :])
                        nc.vector.tensor_mul(
                            pt[:, 512:1024], pt[:, 512:1024], mask1[:, :])
                    nc.tensor.matmul(
                        poa[:, :], v_sb[:, 65 * i0:65 * (i0 + 1)],
                        pt[:, 0:512], start=(ip == 0), stop=(ip == tb))
                    nc.tensor.matmul(
                        pob[:, :], v_sb[:, 65 * i1:65 * (i1 + 1)],
                        pt[:, 512:1024], start=(ip == 0), stop=(ip == tb))
                ob = outp.tile([65, 512], F32, tag="ob")
                nc.vector.tensor_copy(ob[:, :], poa[:, :])
                nc.vector.tensor_add(ob[:, :], pob[:, :], ob[:, :])
                nc.sync.dma_start(
                    out=out_d[:, 512 * tb:512 * (tb + 1)], in_=ob[:, :])

    nc.compile()
    return nc


def make_core_inputs(x, Wk, bk, Wq, bq, Wv, bv, t_full=T):
    """Shard FULL inputs into the 8 per-core input dicts."""
    n_tiles = t_full // 128
    ins = []
    for core in range(N_CORES):
        b, p = core // 2, core % 2
        own = np.concatenate(
            [x[b, 128 * j:128 * (j + 1), :] for j in range(p, n_tiles, 2)],
            axis=0)
        # mask[m][r, c]: s-tile (local parity m, abs tile 4tb+2m+p) vs query
        # sub-tile c//128 (abs tile 4tb + SUB2ABS[c//128]); valid iff s <= t
        masks = np.zeros((2, 128, 512), np.float32)
        rr = np.arange(128)[:, None]
        for m in (0, 1):
            for sub in range(4):
                cz = np.arange(128)[None, :]
                s_abs = 128 * (2 * m + p) + rr
                t_abs = 128 * SUB2ABS[sub] + cz
                masks[m, :, 128 * sub:128 * (sub + 1)] = (s_abs <= t_abs)
        ins.append({
            "x_own": np.ascontiguousarray(own, np.float32),
            "wk": np.asarray(Wk, np.float32), "wq": np.asarray(Wq, np.float32),
            "wv": np.asarray(Wv, np.float32),
            "bk": np.asarray(bk, np.float32), "bq": np.asarray(bq, np.float32),
            "bv": np.asarray(bv, np.float32),
            "masks": masks.astype(ml_dtypes.bfloat16),
        })
    return ins


def _col_perm(t_full):
    """stored column -> absolute t index (same for every core)."""
    perm = np.empty(t_full, np.int64)
    for tb in range(t_full // 512):
        for sub in range(4):
            a = 128 * (4 * tb + SUB2ABS[sub])
            s = 512 * tb + 128 * sub
            perm[s:s + 128] = np.arange(a, a + 128)
    return perm


def combine_outputs(parts, t_full=T):
    """parts: list of 8 arrays [H+1, t_full] -> full output [B, t_full, H]."""
    perm = _col_perm(t_full)
    out = np.empty((B, t_full, H), np.float32)
    for b in range(B):
        acc = parts[2 * b] + parts[2 * b + 1]
        res = acc[:H, :] / acc[H:H + 1, :]
        out[b][perm] = res.T
    return out


_NC_CACHE = {}


def kernel(x, Wk, bk, Wq, bq, Wv, bv):
    x = np.asarray(x, np.float32)
    t_full = x.shape[1]
    if t_full not in _NC_CACHE:
        _NC_CACHE[t_full] = build_kernel(t_full)
    nc = _NC_CACHE[t_full]
    ins = make_core_inputs(x, Wk, bk, Wq, bq, Wv, bv, t_full)
    res = run_bass_kernel_spmd(nc, ins, list(range(N_CORES)))
    parts = [res.results[i]["out_part"] for i in range(N_CORES)]
    return combine_outputs(parts, t_full)


if __name__ == "__main__":
    rng = np.random.default_rng(0)
    x = rng.standard_normal((B, T, C), dtype=np.float32)
    Wk = rng.standard_normal((C, H), dtype=np.float32) * SCALE
    Wq = rng.standard_normal((C, H), dtype=np.float32) * SCALE
    Wv = rng.standard_normal((C, H), dtype=np.float32) * SCALE
    bk = rng.standard_normal(H).astype(np.float32) * 0.02
    bq = rng.standard_normal(H).astype(np.float32) * 0.02
    bv = rng.standard_normal(H).astype(np.float32) * 0.02
    out = kernel(x=x, Wk=Wk, bk=bk, Wq=Wq, bq=bq, Wv=Wv, bv=bv)
    print(out.shape, out.dtype)
